# revision 2
# baseline (speedup 1.0000x reference)
"""Gaussian duration-attention upsampler on 8 Trainium2 NeuronCores.

out[b,t,:] = (sum_i w[b,i,t] * emb[b,i,:]) / (sum_i w[b,i,t] + eps) + PE[t,:]
  with w[b,i,t] = exp(-(t - c[b,i])^2 / ranges[b,i]^2), c = cumsum(dur) - dur/2.

Strategy:
  - Data-parallel over batch: 32 batches -> 4 per core on 8 cores (SPMD, no
    collectives).
  - The Gaussians are narrow (reach <= sqrt(30)*4.5 ~ 25 frames), so W is
    banded: for each 512-frame output chunk only a contiguous window of
    <= 128*nkc tokens matters (max span on this data: ~80 -> nkc=1). The host
    picks the window starts from c (cheap O(B*T_in) preprocessing) and
    gathers the embedding rows per window.
  - W-gen on ScalarE in ONE pass: the HW activation table has Derivative_Erf,
    and erf'(x) = (2/sqrt(pi)) exp(-x^2) -- exactly the needed Gaussian up to
    a constant kappa that cancels in the normalization (the sacrificial eps
    row is scaled by the same kappa). w~ = Derivative_Erf(sqa*t' - sqa*c')
    with per-partition scale/bias APs, one op per 512-frame window. This
    replaces the old two-pass Square+Exp scheme (46us -> 22us of ScalarE).
  - TensorE: [W^T] @ [E|1] in bf16 per 128-frame sub-chunk (N=257; the ones
    column yields the normalizer s~ in PSUM column 256). One window row is a
    constant-eps token (sqa=0, bias=sqrt(-ln eps), zero embedding row), so
    the matmul produces kappa*(s+eps) directly and one merged VectorE
    reciprocal per (b,j) [128,4] off PSUM covers all 4 sub-chunks.
  - Postprocess per sub-chunk, split across three engines: VectorE
    scalar_tensor_tensor fuses out = (U * r) + PE (PSUM read + normalize +
    PE-add in one pass) for most sub-chunks; ScalarE drains the rest with a
    Copy-with-scale (bias stays immediate - cheaper than Identity), and the
    GpSimd/Pool engine (which cannot touch PSUM but can add bf16 in SBUF)
    does the PE-add for the ScalarE-drained sub-chunks.
  - PSUM is allocated as one 4-bank tile per (b,j) (pool bufs=2); output
    DMAs ride the Sync-engine HWDGE queue (GpSimd SWDGE descriptor gen was
    ~13us of Pool time; Sync was nearly idle), inputs also ride Sync.
  - Output frames are permuted (t = 512j + 4q + sub on PSUM partition q) so
    each partition's staged row is 1024 contiguous DRAM elements -> clean
    2KB output DMA descriptors.
  - The last quads shift one extra sub-chunk to ScalarE (n_act=2): once
    W-generation runs dry near the end, ScalarE has slack there.
"""

import numpy as np
import ml_dtypes

import concourse.bacc as bacc
import concourse.mybir as mybir
import concourse.tile as tile
from concourse.bass_utils import run_bass_kernel_spmd

BF16 = ml_dtypes.bfloat16

B, T_IN, D, T_OUT = 32, 512, 256, 4096
EPS = 1e-6
N_CORES = 8
BL = B // N_CORES          # batches per core
NJ = T_OUT // 512          # 512-frame output chunks per batch
CW = 512                   # chunk width (frames)
KW = 128                   # window tokens per k-chunk
TH = 30.0                  # exp(-30) ~ 1e-13: banding threshold

F32 = mybir.dt.float32
BF = mybir.dt.bfloat16

_CACHE = {}


def _pe_table():
    pos = np.arange(T_OUT, dtype=np.float32)[:, None]
    div = np.exp(np.arange(0, D, 2, dtype=np.float32) * (-np.log(10000.0) / D))
    pe = np.zeros((T_OUT, D), np.float32)
    pe[:, 0::2] = np.sin(pos * div)
    pe[:, 1::2] = np.cos(pos * div)
    return pe


N_ACT_SUBS = 1   # sub-chunks drained on ScalarE (rest: VectorE stt)
PAIR = 2         # windows grouped per quad (j-pair)


def _build(nkc):
    """Build + schedule the SPMD bass graph for nkc 128-token k-chunks."""
    nc = bacc.Bacc(
        "TRN2",
        target_bir_lowering=False,
        debug=False,
        enable_asserts=False,
        num_devices=N_CORES,
    )
    eg_d = nc.dram_tensor("eg", (BL, 128, nkc, NJ, 257), BF, kind="ExternalInput")
    # params: [sqa (BL*nkc*NJ) | nsqac (BL*nkc*NJ) | iota (CW)] per partition
    NP = BL * nkc * NJ
    par_d = nc.dram_tensor("par", (128, 2 * NP + CW), F32, kind="ExternalInput")
    pe_d = nc.dram_tensor("pe", (128, NJ * 4 * D), BF, kind="ExternalInput")
    out_d = nc.dram_tensor("out", (BL, T_OUT, D), BF, kind="ExternalOutput")
    # frame t = 512j + 4q + sub lives on partition q, free offset sub*D + d
    out_v2 = out_d[:].rearrange("b (j q s) d -> b q j (s d)", j=NJ, q=128, s=4)

    DErf = mybir.ActivationFunctionType.Derivative_Erf
    Cp = mybir.ActivationFunctionType.Copy
    ADD = mybir.AluOpType.add
    MUL = mybir.AluOpType.mult

    with tile.TileContext(nc) as tc:
        with (
            tc.tile_pool(name="const", bufs=1) as cp,
            tc.tile_pool(name="eg", bufs=BL) as egp,
            tc.tile_pool(name="w", bufs=4) as wp,
            tc.tile_pool(name="ps", bufs=2, space="PSUM") as psp,
            tc.tile_pool(name="rr", bufs=12) as rp,
            tc.tile_pool(name="un", bufs=6) as unp,
            tc.tile_pool(name="ob", bufs=8) as obp,
        ):
            # dummy activation with no DMA deps: forces the ACT table load to
            # the head of the Scalar queue, overlapping it with input DMAs
            dmy = cp.tile([128, 8], F32)
            nc.gpsimd.memset(dmy[:], 0.0)
            zb = dmy[:, 0:1]   # explicit zero bias (avoids const-pool loads)
            dmy2 = cp.tile([128, 8], F32)
            nc.scalar.activation(dmy2[:], dmy[:], DErf, bias=zb)

            par_sb = cp.tile([128, 2 * NP + CW], F32)
            nc.sync.dma_start(par_sb[:], par_d[:])
            sqa_sb = par_sb[:, 0:NP].rearrange("p (b k j) -> p b k j", b=BL, k=nkc)
            nsqac_sb = par_sb[:, NP : 2 * NP].rearrange(
                "p (b k j) -> p b k j", b=BL, k=nkc
            )
            iota_sb = par_sb[:, 2 * NP :]
            eg_sbs = [
                egp.tile([128, nkc, NJ, 257], BF, name=f"egt{b}", tag=f"eg{b}")
                for b in range(BL)
            ]
            nc.sync.dma_start(eg_sbs[0][:, :, 0:1, :], eg_d[0][:, :, 0:1, :])
            nc.sync.dma_start(eg_sbs[0][:, :, 1:, :], eg_d[0][:, :, 1:, :])
            pe_sb = cp.tile([128, NJ * 4 * D], BF)
            half = NJ * 2 * D
            nc.sync.dma_start(pe_sb[:, :half], pe_d[:, :half])
            nc.sync.dma_start(pe_sb[:, half:], pe_d[:, half:])
            for b in range(1, BL):
                nc.sync.dma_start(eg_sbs[b][:], eg_d[b])

            PW = PAIR * nkc * CW     # W columns per quad-group
            quads = [
                (b, list(range(j0, min(j0 + PAIR, NJ))))
                for b in range(BL)
                for j0 in range(0, NJ, PAIR)
            ]

            def wgen(q):
                """w~ = kappa*exp(-(sqa*t' - sqa*c')^2) via Derivative_Erf
                (per-partition scale+bias on ScalarE, one op per window)."""
                b, js = q
                w_b = wp.tile([128, PW], BF, tag="w")
                for ji, j in enumerate(js):
                    for kc in range(nkc):
                        o = (ji * nkc + kc) * CW
                        nc.scalar.activation(
                            w_b[:, o : o + CW], iota_sb, DErf,
                            bias=nsqac_sb[:, b, kc, j : j + 1],
                            scale=sqa_sb[:, b, kc, j : j + 1],
                        )
                return w_b

            gidx = 0
            w_next = wgen(quads[0])
            for qi, (b, js) in enumerate(quads):
                w_b = w_next
                if qi + 1 < len(quads):
                    # next quad's W-gen goes to the ScalarE queue ahead of
                    # this quad's drain-copies (which wait on matmuls)
                    w_next = wgen(quads[qi + 1])
                n_act = 2 if qi >= 13 else N_ACT_SUBS
                lo = 4 - n_act
                j0 = js[0]
                ob_q = obp.tile([128, 2 * 4 * D], BF, name=f"obq{qi}", tag="ob")
                un_q = (unp.tile([128, 2, 2 * D], BF, name=f"unq{qi}", tag="un")
                        if n_act else None)
                for ji, j in enumerate(js):
                    ps = psp.tile([128, 4, 512], F32, name=f"ps{b}_{j}", tag="ps")
                    for sub in range(4):
                        for kc in range(nkc):
                            o = (ji * nkc + kc) * CW + sub * 128
                            nc.tensor.matmul(
                                ps[:, sub, 0:257],
                                w_b[:, o : o + 128],
                                eg_sbs[b][:, kc, j, :],
                                start=(kc == 0),
                                stop=(kc == nkc - 1),
                            )
                    # eps is already inside s~: one window row is a
                    # constant-eps token (sqa=0, bias=sqrt(-ln eps), zero
                    # embedding), so r = 1/(kappa*(s+eps)) is one merged
                    # reciprocal over all 4 sub-chunks
                    r4 = rp.tile([128, 4], F32, name=f"r4_{gidx}", tag="r4")
                    nc.vector.reciprocal(r4[:, 0:4], ps[:, :, 256])
                    gidx += 1
                    for i, sub in enumerate(range(lo, 4)):
                        nc.scalar.activation(
                            un_q[:, ji, i * D : (i + 1) * D],
                            ps[:, sub, 0:D],
                            Cp, scale=r4[:, sub : sub + 1],
                        )
                    for sub in range(lo):
                        g = j * 4 + sub
                        nc.vector.scalar_tensor_tensor(
                            ob_q[:, ji * 1024 + sub * D
                                 : ji * 1024 + (sub + 1) * D],
                            ps[:, sub, 0:D],
                            r4[:, sub : sub + 1],
                            pe_sb[:, g * D : (g + 1) * D],
                            MUL,
                            ADD,
                        )
                # one fused PE-add on the Pool engine + one out-DMA per quad
                obv = ob_q[:].rearrange("p (g x) -> p g x", g=2)
                if n_act:
                    pev = pe_sb[:, j0 * 1024 : (j0 + 2) * 1024].rearrange(
                        "p (g x) -> p g x", g=2)
                    nc.gpsimd.tensor_tensor(
                        obv[:, :, lo * D :],
                        un_q[:, :, : n_act * D],
                        pev[:, :, lo * D :],
                        ADD,
                    )
                nc.sync.dma_start(out_v2[b, :, j0 : j0 + 2], obv[:])

    nc.compile()
    return nc


def kernel(embeddings, durations, ranges, t_out):
    assert int(t_out) == T_OUT
    emb = np.asarray(embeddings, dtype=np.float32)
    dur = np.asarray(durations, dtype=np.float32)[:, :, 0]
    rng = np.asarray(ranges, dtype=np.float32)[:, :, 0]

    # ---- host preprocessing: O(B*T_in) scalars + window selection ----
    c = np.cumsum(dur, axis=1, dtype=np.float32) - 0.5 * dur   # (B, T_IN)
    a = rng.astype(np.float32) ** -2
    reach = np.sqrt(TH) / np.sqrt(a)

    # window starts: tokens whose gaussian reaches into chunk j
    starts = np.zeros((B, NJ), np.int32)
    span_max = 1
    for b in range(B):
        lo_r, hi_r = c[b] - reach[b], c[b] + reach[b]
        for j in range(NJ):
            qual = np.nonzero((lo_r <= CW * j + CW - 1) & (hi_r >= CW * j))[0]
            if len(qual):
                span_max = max(span_max, int(np.ceil((qual[-1] - qual[0] + 2) / KW)))
                starts[b, j] = qual[0]
            else:
                starts[b, j] = 0
    nkc = span_max
    usable = KW * nkc - 1          # last row of the last k-chunk carries eps
    starts = np.minimum(starts, T_IN - usable)
    # coverage assert (windows are contiguous token ranges)
    for b in range(B):
        lo_r, hi_r = c[b] - reach[b], c[b] + reach[b]
        for j in range(NJ):
            qual = np.nonzero((lo_r <= CW * j + CW - 1) & (hi_r >= CW * j))[0]
            if len(qual):
                assert starts[b, j] <= qual[0] and qual[-1] < starts[b, j] + usable

    # gathered per-window tensors
    ea = np.ones((B, T_IN, 257), BF16)
    ea[:, :, :256] = emb.astype(BF16)
    kidx = starts[:, None, :, None] + (
        np.arange(nkc)[None, :, None, None] * KW + np.arange(KW)[None, None, None, :]
    )  # (B, nkc, NJ, KW); the last slot is overwritten by the eps row below
    kidx = np.minimum(kidx, T_IN - 1)
    bidx = np.arange(B)[:, None, None, None]
    eg = ea[bidx, kidx]                       # (B, nkc, NJ, KW, 257)
    eg = eg.transpose(0, 3, 1, 2, 4).copy()   # (B, KW, nkc, NJ, 257)
    cg = c[bidx, kidx]                        # (B, nkc, NJ, KW)
    ag = a[bidx, kidx]
    jgrid = np.arange(NJ, dtype=np.float32)[None, None, :, None] * CW
    sqa_g = np.sqrt(ag)
    sqa = sqa_g.transpose(3, 0, 1, 2).astype(np.float32)                 # (KW,B,nkc,NJ)
    nsqac = (sqa_g * (jgrid - cg)).transpose(3, 0, 1, 2).astype(np.float32)
    # sacrificial eps row: w~ = kappa*exp(-(0*t + sqrt(-ln eps))^2) =
    # kappa*eps for all t (kappa = 2/sqrt(pi) from Derivative_Erf cancels
    # against the same factor in the numerator weights)
    eg[:, 127, -1, :, :] = 0.0
    eg[:, 127, -1, :, 256] = 1.0
    sqa[127, :, -1, :] = 0.0
    nsqac[127, :, -1, :] = np.float32(np.sqrt(-np.log(EPS)))

    # constants: permuted iota (col f of W is frame t' = 4*(f%128) + f//128)
    f = np.arange(CW)
    tperm = (4 * (f % 128) + f // 128).astype(np.float32)
    iota = np.broadcast_to(tperm, (128, CW)).copy()
    # PE in the same permuted layout: tile[q, j*1024 + sub*256 + d] = PE[512j+4q+sub, d]
    pe = _pe_table().reshape(NJ, 128, 4, D).transpose(1, 0, 2, 3).reshape(128, -1)
    pe = pe.astype(BF16)

    key = nkc
    if key not in _CACHE:
        _CACHE[key] = _build(nkc)
    nc = _CACHE[key]

    NP = BL * nkc * NJ
    in_maps = []
    for i in range(N_CORES):
        bs = slice(i * BL, (i + 1) * BL)
        par = np.concatenate(
            [
                sqa[:, bs].reshape(128, NP),
                nsqac[:, bs].reshape(128, NP),
                iota,
            ],
            axis=1,
        ).astype(np.float32)
        in_maps.append({
            "eg": np.ascontiguousarray(eg[bs]),
            "par": np.ascontiguousarray(par),
            "pe": pe,
        })

    res = run_bass_kernel_spmd(nc, in_maps, core_ids=list(range(N_CORES)))
    out = np.concatenate([r["out"] for r in res.results], axis=0)
    return out.astype(np.float32)


# revision 5
# speedup vs baseline: 1.1797x; 1.1797x over previous
"""Gaussian duration-attention upsampler on 8 Trainium2 NeuronCores.

out[b,t,:] = (sum_i w[b,i,t] * emb[b,i,:]) / (sum_i w[b,i,t] + eps) + PE[t,:]
  with w[b,i,t] = exp(-(t - c[b,i])^2 / ranges[b,i]^2), c = cumsum(dur) - dur/2.

Strategy:
  - Data-parallel over batch: 32 batches -> 4 per core on 8 cores (SPMD, no
    collectives).
  - The Gaussians are narrow (reach <= sqrt(30)*4.5 ~ 25 frames), so W is
    banded: for each 512-frame output chunk only a contiguous window of
    <= 128*nkc tokens matters (max span on this data: ~80 -> nkc=1). The host
    picks the window starts from c (cheap O(B*T_in) preprocessing) and
    gathers the embedding rows per window.
  - W-gen on ScalarE in ONE pass: the HW activation table has Derivative_Erf,
    and erf'(x) = (2/sqrt(pi)) exp(-x^2) -- exactly the needed Gaussian up to
    a constant kappa that cancels in the normalization (the sacrificial eps
    row is scaled by the same kappa). w~ = Derivative_Erf(sqa*t' - sqa*c')
    with per-partition scale/bias APs, one op per 512-frame window. This
    replaces the old two-pass Square+Exp scheme (46us -> 22us of ScalarE).
  - TensorE: [W^T] @ [E|1] in bf16 per 128-frame sub-chunk (N=257; the ones
    column yields the normalizer s~ in PSUM column 256). One window row is a
    constant-eps token (sqa=0, bias=sqrt(-ln eps), zero embedding row), so
    the matmul produces kappa*(s+eps) directly and one merged VectorE
    reciprocal per (b,j) [128,4] off PSUM covers all 4 sub-chunks.
  - Postprocess per sub-chunk, split across three engines: VectorE
    scalar_tensor_tensor fuses out = (U * r) + PE (PSUM read + normalize +
    PE-add in one pass) for most sub-chunks; ScalarE drains the rest with a
    Copy-with-scale (bias stays immediate - cheaper than Identity), and the
    GpSimd/Pool engine (which cannot touch PSUM but can add bf16 in SBUF)
    does the PE-add for the ScalarE-drained sub-chunks.
  - PSUM is allocated as one 4-bank tile per (b,j) (pool bufs=2); output
    DMAs ride the Sync-engine HWDGE queue (GpSimd SWDGE descriptor gen was
    ~13us of Pool time; Sync was nearly idle), inputs also ride Sync.
  - Output frames are permuted (t = 512j + 4q + sub on PSUM partition q) so
    each partition's staged row is 1024 contiguous DRAM elements -> clean
    2KB output DMA descriptors.
  - The last quads shift one extra sub-chunk to ScalarE (n_act=2): once
    W-generation runs dry near the end, ScalarE has slack there.
"""

import numpy as np
import ml_dtypes

import concourse.bacc as bacc
import concourse.mybir as mybir
import concourse.tile as tile
from concourse.bass_utils import run_bass_kernel_spmd

BF16 = ml_dtypes.bfloat16

B, T_IN, D, T_OUT = 32, 512, 256, 4096
EPS = 1e-6
N_CORES = 8
BL = B // N_CORES          # batches per core
NJ = T_OUT // 512          # 512-frame output chunks per batch
CW = 512                   # chunk width (frames)
KW = 128                   # window tokens per k-chunk
TH = 30.0                  # exp(-30) ~ 1e-13: banding threshold

F32 = mybir.dt.float32
BF = mybir.dt.bfloat16

_CACHE = {}


def _pe_table():
    pos = np.arange(T_OUT, dtype=np.float32)[:, None]
    div = np.exp(np.arange(0, D, 2, dtype=np.float32) * (-np.log(10000.0) / D))
    pe = np.zeros((T_OUT, D), np.float32)
    pe[:, 0::2] = np.sin(pos * div)
    pe[:, 1::2] = np.cos(pos * div)
    return pe


N_ACT_SUBS = 1   # sub-chunks drained on ScalarE (rest: VectorE stt)
PAIR = 2         # windows grouped per quad (j-pair)


def _build(nkc):
    """Build + schedule the SPMD bass graph for nkc 128-token k-chunks."""
    nc = bacc.Bacc(
        "TRN2",
        target_bir_lowering=False,
        debug=False,
        enable_asserts=False,
        num_devices=N_CORES,
    )
    eg_d = nc.dram_tensor("eg", (BL, 128, nkc, NJ, 257), BF, kind="ExternalInput")
    # params: [sqa (BL*nkc*NJ) | nsqac (BL*nkc*NJ) | iota (CW)] per partition
    NP = BL * nkc * NJ
    par_d = nc.dram_tensor("par", (128, 2 * NP + CW), F32, kind="ExternalInput")
    pe_d = nc.dram_tensor("pe", (128, NJ * 4 * D), BF, kind="ExternalInput")
    out_d = nc.dram_tensor("out", (BL, T_OUT, D), BF, kind="ExternalOutput")
    # frame t = 512j + 4q + sub lives on partition q, free offset sub*D + d
    out_v2 = out_d[:].rearrange("b (j q s) d -> b q j (s d)", j=NJ, q=128, s=4)

    DErf = mybir.ActivationFunctionType.Derivative_Erf
    Cp = mybir.ActivationFunctionType.Copy
    ADD = mybir.AluOpType.add
    MUL = mybir.AluOpType.mult

    with tile.TileContext(nc) as tc:
        with (
            tc.tile_pool(name="const", bufs=1) as cp,
            tc.tile_pool(name="eg", bufs=BL) as egp,
            tc.tile_pool(name="w", bufs=4) as wp,
            tc.tile_pool(name="ps", bufs=4, space="PSUM") as psp,
            tc.tile_pool(name="rr", bufs=12) as rp,
            tc.tile_pool(name="un", bufs=6) as unp,
            tc.tile_pool(name="ob", bufs=8) as obp,
        ):
            # dummy activation with no DMA deps: forces the ACT table load to
            # the head of the Scalar queue, overlapping it with input DMAs
            dmy = cp.tile([128, 8], F32)
            nc.gpsimd.memset(dmy[:], 0.0)
            zb = dmy[:, 0:1]   # explicit zero bias (avoids const-pool loads)
            dmy2 = cp.tile([128, 8], F32)
            nc.scalar.activation(dmy2[:], dmy[:], DErf, bias=zb)

            par_sb = cp.tile([128, 2 * NP + CW], F32)
            nc.sync.dma_start(par_sb[:], par_d[:])
            sqa_sb = par_sb[:, 0:NP].rearrange("p (b k j) -> p b k j", b=BL, k=nkc)
            nsqac_sb = par_sb[:, NP : 2 * NP].rearrange(
                "p (b k j) -> p b k j", b=BL, k=nkc
            )
            iota_sb = par_sb[:, 2 * NP :]
            eg_sbs = [
                egp.tile([128, nkc, NJ, 257], BF, name=f"egt{b}", tag=f"eg{b}")
                for b in range(BL)
            ]
            nc.sync.dma_start(eg_sbs[0][:, :, 0:1, :], eg_d[0][:, :, 0:1, :])
            nc.sync.dma_start(eg_sbs[0][:, :, 1:, :], eg_d[0][:, :, 1:, :])
            pe_sb = cp.tile([128, NJ * 4 * D], BF)
            half = NJ * 2 * D
            nc.sync.dma_start(pe_sb[:, :half], pe_d[:, :half])
            nc.sync.dma_start(pe_sb[:, half:], pe_d[:, half:])
            for b in range(1, BL):
                nc.sync.dma_start(eg_sbs[b][:], eg_d[b])

            PW = PAIR * nkc * CW     # W columns per quad-group
            quads = [
                (b, list(range(j0, min(j0 + PAIR, NJ))))
                for b in range(BL)
                for j0 in range(0, NJ, PAIR)
            ]

            def wgen(q):
                """w~ = kappa*exp(-(sqa*t' - sqa*c')^2) via Derivative_Erf
                (per-partition scale+bias on ScalarE, one op per window)."""
                b, js = q
                w_b = wp.tile([128, PW], BF, tag="w")
                for ji, j in enumerate(js):
                    for kc in range(nkc):
                        o = (ji * nkc + kc) * CW
                        nc.scalar.activation(
                            w_b[:, o : o + CW], iota_sb, DErf,
                            bias=nsqac_sb[:, b, kc, j : j + 1],
                            scale=sqa_sb[:, b, kc, j : j + 1],
                        )
                return w_b

            gidx = 0
            w_next = wgen(quads[0])
            for qi, (b, js) in enumerate(quads):
                w_b = w_next
                if qi + 1 < len(quads):
                    # next quad's W-gen goes to the ScalarE queue ahead of
                    # this quad's drain-copies (which wait on matmuls)
                    w_next = wgen(quads[qi + 1])
                n_act = 2 if qi >= 14 else N_ACT_SUBS
                lo = 4 - n_act
                j0 = js[0]
                ob_q = obp.tile([128, 2 * 4 * D], BF, name=f"obq{qi}", tag="ob")
                un_q = (unp.tile([128, 2, 2 * D], BF, name=f"unq{qi}", tag="un")
                        if n_act else None)
                for ji, j in enumerate(js):
                    # two 2-bank PSUM tiles per (b,j): psA (subs 0,1) drains
                    # fast on VectorE and recycles early for the next group's
                    # matmuls; psB (subs 2,3) carries the ScalarE-drained
                    # sub-chunks, whose copies queue behind next-quad W-gen
                    psA = psp.tile([128, 2, 512], F32, name=f"psA{b}_{j}",
                                   tag="ps")
                    psB = psp.tile([128, 2, 512], F32, name=f"psB{b}_{j}",
                                   tag="ps")
                    halves = [psA, psA, psB, psB]
                    for sub in range(4):
                        for kc in range(nkc):
                            o = (ji * nkc + kc) * CW + sub * 128
                            nc.tensor.matmul(
                                halves[sub][:, sub % 2, 0:257],
                                w_b[:, o : o + 128],
                                eg_sbs[b][:, kc, j, :],
                                start=(kc == 0),
                                stop=(kc == nkc - 1),
                            )
                    # eps is already inside s~: one window row is a
                    # constant-eps token (sqa=0, bias=sqrt(-ln eps), zero
                    # embedding), so r = 1/(kappa*(s+eps)) is a plain
                    # reciprocal straight off PSUM
                    r4 = rp.tile([128, 4], F32, name=f"r4_{gidx}", tag="r4")
                    nc.vector.reciprocal(r4[:, 0:2], psA[:, :, 256])
                    gidx += 1
                    for sub in range(2):
                        g = j * 4 + sub
                        nc.vector.scalar_tensor_tensor(
                            ob_q[:, ji * 1024 + sub * D
                                 : ji * 1024 + (sub + 1) * D],
                            halves[sub][:, sub % 2, 0:D],
                            r4[:, sub : sub + 1],
                            pe_sb[:, g * D : (g + 1) * D],
                            MUL,
                            ADD,
                        )
                    nc.vector.reciprocal(r4[:, 2:4], psB[:, :, 256])
                    for i, sub in enumerate(range(lo, 4)):
                        nc.scalar.activation(
                            un_q[:, ji, i * D : (i + 1) * D],
                            halves[sub][:, sub % 2, 0:D],
                            Cp, scale=r4[:, sub : sub + 1],
                        )
                    for sub in range(2, lo):
                        g = j * 4 + sub
                        nc.vector.scalar_tensor_tensor(
                            ob_q[:, ji * 1024 + sub * D
                                 : ji * 1024 + (sub + 1) * D],
                            halves[sub][:, sub % 2, 0:D],
                            r4[:, sub : sub + 1],
                            pe_sb[:, g * D : (g + 1) * D],
                            MUL,
                            ADD,
                        )
                # one fused PE-add on the Pool engine + one out-DMA per quad
                obv = ob_q[:].rearrange("p (g x) -> p g x", g=2)
                if n_act:
                    pev = pe_sb[:, j0 * 1024 : (j0 + 2) * 1024].rearrange(
                        "p (g x) -> p g x", g=2)
                    nc.gpsimd.tensor_tensor(
                        obv[:, :, lo * D :],
                        un_q[:, :, : n_act * D],
                        pev[:, :, lo * D :],
                        ADD,
                    )
                nc.sync.dma_start(out_v2[b, :, j0 : j0 + 2], obv[:])

    nc.compile()
    return nc


def kernel(embeddings, durations, ranges, t_out):
    assert int(t_out) == T_OUT
    emb = np.asarray(embeddings, dtype=np.float32)
    dur = np.asarray(durations, dtype=np.float32)[:, :, 0]
    rng = np.asarray(ranges, dtype=np.float32)[:, :, 0]

    # ---- host preprocessing: O(B*T_in) scalars + window selection ----
    c = np.cumsum(dur, axis=1, dtype=np.float32) - 0.5 * dur   # (B, T_IN)
    a = rng.astype(np.float32) ** -2
    reach = np.sqrt(TH) / np.sqrt(a)

    # window starts: tokens whose gaussian reaches into chunk j
    starts = np.zeros((B, NJ), np.int32)
    span_max = 1
    for b in range(B):
        lo_r, hi_r = c[b] - reach[b], c[b] + reach[b]
        for j in range(NJ):
            qual = np.nonzero((lo_r <= CW * j + CW - 1) & (hi_r >= CW * j))[0]
            if len(qual):
                span_max = max(span_max, int(np.ceil((qual[-1] - qual[0] + 2) / KW)))
                starts[b, j] = qual[0]
            else:
                starts[b, j] = 0
    nkc = span_max
    usable = KW * nkc - 1          # last row of the last k-chunk carries eps
    starts = np.minimum(starts, T_IN - usable)
    # coverage assert (windows are contiguous token ranges)
    for b in range(B):
        lo_r, hi_r = c[b] - reach[b], c[b] + reach[b]
        for j in range(NJ):
            qual = np.nonzero((lo_r <= CW * j + CW - 1) & (hi_r >= CW * j))[0]
            if len(qual):
                assert starts[b, j] <= qual[0] and qual[-1] < starts[b, j] + usable

    # gathered per-window tensors
    ea = np.ones((B, T_IN, 257), BF16)
    ea[:, :, :256] = emb.astype(BF16)
    kidx = starts[:, None, :, None] + (
        np.arange(nkc)[None, :, None, None] * KW + np.arange(KW)[None, None, None, :]
    )  # (B, nkc, NJ, KW); the last slot is overwritten by the eps row below
    kidx = np.minimum(kidx, T_IN - 1)
    bidx = np.arange(B)[:, None, None, None]
    eg = ea[bidx, kidx]                       # (B, nkc, NJ, KW, 257)
    eg = eg.transpose(0, 3, 1, 2, 4).copy()   # (B, KW, nkc, NJ, 257)
    cg = c[bidx, kidx]                        # (B, nkc, NJ, KW)
    ag = a[bidx, kidx]
    jgrid = np.arange(NJ, dtype=np.float32)[None, None, :, None] * CW
    sqa_g = np.sqrt(ag)
    sqa = sqa_g.transpose(3, 0, 1, 2).astype(np.float32)                 # (KW,B,nkc,NJ)
    nsqac = (sqa_g * (jgrid - cg)).transpose(3, 0, 1, 2).astype(np.float32)
    # sacrificial eps row: w~ = kappa*exp(-(0*t + sqrt(-ln eps))^2) =
    # kappa*eps for all t (kappa = 2/sqrt(pi) from Derivative_Erf cancels
    # against the same factor in the numerator weights)
    eg[:, 127, -1, :, :] = 0.0
    eg[:, 127, -1, :, 256] = 1.0
    sqa[127, :, -1, :] = 0.0
    nsqac[127, :, -1, :] = np.float32(np.sqrt(-np.log(EPS)))

    # constants: permuted iota (col f of W is frame t' = 4*(f%128) + f//128)
    f = np.arange(CW)
    tperm = (4 * (f % 128) + f // 128).astype(np.float32)
    iota = np.broadcast_to(tperm, (128, CW)).copy()
    # PE in the same permuted layout: tile[q, j*1024 + sub*256 + d] = PE[512j+4q+sub, d]
    pe = _pe_table().reshape(NJ, 128, 4, D).transpose(1, 0, 2, 3).reshape(128, -1)
    pe = pe.astype(BF16)

    key = nkc
    if key not in _CACHE:
        _CACHE[key] = _build(nkc)
    nc = _CACHE[key]

    NP = BL * nkc * NJ
    in_maps = []
    for i in range(N_CORES):
        bs = slice(i * BL, (i + 1) * BL)
        par = np.concatenate(
            [
                sqa[:, bs].reshape(128, NP),
                nsqac[:, bs].reshape(128, NP),
                iota,
            ],
            axis=1,
        ).astype(np.float32)
        in_maps.append({
            "eg": np.ascontiguousarray(eg[bs]),
            "par": np.ascontiguousarray(par),
            "pe": pe,
        })

    res = run_bass_kernel_spmd(nc, in_maps, core_ids=list(range(N_CORES)))
    out = np.concatenate([r["out"] for r in res.results], axis=0)
    return out.astype(np.float32)


# revision 7
# speedup vs baseline: 1.1871x; 1.0063x over previous
"""Gaussian duration-attention upsampler on 8 Trainium2 NeuronCores.

out[b,t,:] = (sum_i w[b,i,t] * emb[b,i,:]) / (sum_i w[b,i,t] + eps) + PE[t,:]
  with w[b,i,t] = exp(-(t - c[b,i])^2 / ranges[b,i]^2), c = cumsum(dur) - dur/2.

Strategy:
  - Data-parallel over batch: 32 batches -> 4 per core on 8 cores (SPMD, no
    collectives).
  - The Gaussians are narrow (reach <= sqrt(30)*4.5 ~ 25 frames), so W is
    banded: for each 512-frame output chunk only a contiguous window of
    <= 128*nkc tokens matters (max span on this data: ~80 -> nkc=1). The host
    picks the window starts from c (cheap O(B*T_in) preprocessing) and
    gathers the embedding rows per window.
  - W-gen on ScalarE in ONE pass: the HW activation table has Derivative_Erf,
    and erf'(x) = (2/sqrt(pi)) exp(-x^2) -- exactly the needed Gaussian up to
    a constant kappa that cancels in the normalization (the sacrificial eps
    row is scaled by the same kappa). w~ = Derivative_Erf(sqa*t' - sqa*c')
    with per-partition scale/bias APs, one op per 512-frame window. This
    replaces the old two-pass Square+Exp scheme (46us -> 22us of ScalarE).
  - TensorE: [W^T] @ [E|1] in bf16 per 128-frame sub-chunk (N=257; the ones
    column yields the normalizer s~ in PSUM column 256). One window row is a
    constant-eps token (sqa=0, bias=sqrt(-ln eps), zero embedding row), so
    the matmul produces kappa*(s+eps) directly and one merged VectorE
    reciprocal per (b,j) [128,4] off PSUM covers all 4 sub-chunks.
  - Postprocess per sub-chunk, split across three engines: VectorE
    scalar_tensor_tensor fuses out = (U * r) + PE (PSUM read + normalize +
    PE-add in one pass) for most sub-chunks; ScalarE drains the rest with a
    Copy-with-scale (bias stays immediate - cheaper than Identity), and the
    GpSimd/Pool engine (which cannot touch PSUM but can add bf16 in SBUF)
    does the PE-add for the ScalarE-drained sub-chunks.
  - PSUM is allocated as one 4-bank tile per (b,j) (pool bufs=2); output
    DMAs ride the Sync-engine HWDGE queue (GpSimd SWDGE descriptor gen was
    ~13us of Pool time; Sync was nearly idle), inputs also ride Sync.
  - Output frames are permuted (t = 512j + 4q + sub on PSUM partition q) so
    each partition's staged row is 1024 contiguous DRAM elements -> clean
    2KB output DMA descriptors.
  - The last quads shift one extra sub-chunk to ScalarE (n_act=2): once
    W-generation runs dry near the end, ScalarE has slack there.
"""

import numpy as np
import ml_dtypes

import concourse.bacc as bacc
import concourse.mybir as mybir
import concourse.tile as tile
from concourse.bass_utils import run_bass_kernel_spmd

BF16 = ml_dtypes.bfloat16

B, T_IN, D, T_OUT = 32, 512, 256, 4096
EPS = 1e-6
N_CORES = 8
BL = B // N_CORES          # batches per core
NJ = T_OUT // 512          # 512-frame output chunks per batch
CW = 512                   # chunk width (frames)
KW = 128                   # window tokens per k-chunk
TH = 30.0                  # exp(-30) ~ 1e-13: banding threshold

F32 = mybir.dt.float32
BF = mybir.dt.bfloat16

_CACHE = {}


def _pe_table():
    pos = np.arange(T_OUT, dtype=np.float32)[:, None]
    div = np.exp(np.arange(0, D, 2, dtype=np.float32) * (-np.log(10000.0) / D))
    pe = np.zeros((T_OUT, D), np.float32)
    pe[:, 0::2] = np.sin(pos * div)
    pe[:, 1::2] = np.cos(pos * div)
    return pe


N_ACT_SUBS = 1   # sub-chunks drained on ScalarE (rest: VectorE stt)
PAIR = 2         # windows grouped per quad (j-pair)


def _build(nkc):
    """Build + schedule the SPMD bass graph for nkc 128-token k-chunks."""
    nc = bacc.Bacc(
        "TRN2",
        target_bir_lowering=False,
        debug=False,
        enable_asserts=False,
        num_devices=N_CORES,
    )
    eg_d = nc.dram_tensor("eg", (BL, 128, nkc, NJ, 257), BF, kind="ExternalInput")
    # params: [sqa (BL*nkc*NJ) | nsqac (BL*nkc*NJ) | iota (CW)] per partition
    NP = BL * nkc * NJ
    par_d = nc.dram_tensor("par", (128, 2 * NP + CW), F32, kind="ExternalInput")
    pe_d = nc.dram_tensor("pe", (128, NJ * 4 * D), BF, kind="ExternalInput")
    out_d = nc.dram_tensor("out", (BL, T_OUT, D), BF, kind="ExternalOutput")
    # frame t = 512j + 4q + sub lives on partition q, free offset sub*D + d
    out_v2 = out_d[:].rearrange("b (j q s) d -> b q j (s d)", j=NJ, q=128, s=4)

    DErf = mybir.ActivationFunctionType.Derivative_Erf
    Cp = mybir.ActivationFunctionType.Copy
    ADD = mybir.AluOpType.add
    MUL = mybir.AluOpType.mult

    with tile.TileContext(nc) as tc:
        with (
            tc.tile_pool(name="const", bufs=1) as cp,
            tc.tile_pool(name="eg", bufs=BL) as egp,
            tc.tile_pool(name="w", bufs=4) as wp,
            tc.tile_pool(name="ps", bufs=4, space="PSUM") as psp,
            tc.tile_pool(name="rr", bufs=12) as rp,
            tc.tile_pool(name="un", bufs=6) as unp,
            tc.tile_pool(name="ob", bufs=8) as obp,
        ):
            # dummy activation with no DMA deps: forces the ACT table load to
            # the head of the Scalar queue, overlapping it with input DMAs
            dmy = cp.tile([128, 8], F32)
            nc.gpsimd.memset(dmy[:], 0.0)
            zb = dmy[:, 0:1]   # explicit zero bias (avoids const-pool loads)
            dmy2 = cp.tile([128, 8], F32)
            nc.scalar.activation(dmy2[:], dmy[:], DErf, bias=zb)

            par_sb = cp.tile([128, 2 * NP + CW], F32)
            nc.sync.dma_start(par_sb[:], par_d[:])
            sqa_sb = par_sb[:, 0:NP].rearrange("p (b k j) -> p b k j", b=BL, k=nkc)
            nsqac_sb = par_sb[:, NP : 2 * NP].rearrange(
                "p (b k j) -> p b k j", b=BL, k=nkc
            )
            iota_sb = par_sb[:, 2 * NP :]
            eg_sbs = [
                egp.tile([128, nkc, NJ, 257], BF, name=f"egt{b}", tag=f"eg{b}")
                for b in range(BL)
            ]
            pe_sb = cp.tile([128, NJ * 4 * D], BF)
            # input DMA order follows first-use time: eg batches interleave
            # with PE-table quarters so neither Tensor (eg) nor Vector (pe)
            # stalls at startup
            Q = NJ * D  # pe columns per 2 output chunks
            nc.sync.dma_start(eg_sbs[0][:, :, 0:1, :], eg_d[0][:, :, 0:1, :])
            nc.sync.dma_start(eg_sbs[0][:, :, 1:4, :], eg_d[0][:, :, 1:4, :])
            nc.sync.dma_start(pe_sb[:, : 2 * Q // 4], pe_d[:, : 2 * Q // 4])
            nc.sync.dma_start(eg_sbs[0][:, :, 4:, :], eg_d[0][:, :, 4:, :])
            nc.sync.dma_start(eg_sbs[1][:], eg_d[1])
            nc.sync.dma_start(pe_sb[:, 2 * Q // 4 : Q], pe_d[:, 2 * Q // 4 : Q])
            nc.sync.dma_start(pe_sb[:, Q:], pe_d[:, Q:])
            nc.sync.dma_start(eg_sbs[2][:], eg_d[2])
            nc.sync.dma_start(eg_sbs[3][:], eg_d[3])

            PW = PAIR * nkc * CW     # W columns per quad-group
            quads = [
                (b, list(range(j0, min(j0 + PAIR, NJ))))
                for b in range(BL)
                for j0 in range(0, NJ, PAIR)
            ]

            def wgen(q):
                """w~ = kappa*exp(-(sqa*t' - sqa*c')^2) via Derivative_Erf
                (per-partition scale+bias on ScalarE, one op per window)."""
                b, js = q
                w_b = wp.tile([128, PW], BF, tag="w")
                for ji, j in enumerate(js):
                    for kc in range(nkc):
                        o = (ji * nkc + kc) * CW
                        nc.scalar.activation(
                            w_b[:, o : o + CW], iota_sb, DErf,
                            bias=nsqac_sb[:, b, kc, j : j + 1],
                            scale=sqa_sb[:, b, kc, j : j + 1],
                        )
                return w_b

            gidx = 0
            w_next = wgen(quads[0])
            for qi, (b, js) in enumerate(quads):
                w_b = w_next
                if qi + 1 < len(quads):
                    # next quad's W-gen goes to the ScalarE queue ahead of
                    # this quad's drain-copies (which wait on matmuls)
                    w_next = wgen(quads[qi + 1])
                n_act = 2 if qi >= 14 else N_ACT_SUBS
                lo = 4 - n_act
                j0 = js[0]
                ob_q = obp.tile([128, 2 * 4 * D], BF, name=f"obq{qi}", tag="ob")
                un_q = (unp.tile([128, 2, 2 * D], BF, name=f"unq{qi}", tag="un")
                        if n_act else None)
                for ji, j in enumerate(js):
                    # two 2-bank PSUM tiles per (b,j): psA (subs 0,1) drains
                    # fast on VectorE and recycles early for the next group's
                    # matmuls; psB (subs 2,3) carries the ScalarE-drained
                    # sub-chunks, whose copies queue behind next-quad W-gen
                    psA = psp.tile([128, 2, 512], F32, name=f"psA{b}_{j}",
                                   tag="ps")
                    psB = psp.tile([128, 2, 512], F32, name=f"psB{b}_{j}",
                                   tag="ps")
                    halves = [psA, psA, psB, psB]
                    for sub in range(4):
                        for kc in range(nkc):
                            o = (ji * nkc + kc) * CW + sub * 128
                            nc.tensor.matmul(
                                halves[sub][:, sub % 2, 0:257],
                                w_b[:, o : o + 128],
                                eg_sbs[b][:, kc, j, :],
                                start=(kc == 0),
                                stop=(kc == nkc - 1),
                            )
                    # eps is already inside s~: one window row is a
                    # constant-eps token (sqa=0, bias=sqrt(-ln eps), zero
                    # embedding), so r = 1/(kappa*(s+eps)) is a plain
                    # reciprocal straight off PSUM
                    r4 = rp.tile([128, 4], F32, name=f"r4_{gidx}", tag="r4")
                    nc.vector.reciprocal(r4[:, 0:2], psA[:, :, 256])
                    gidx += 1
                    for sub in range(2):
                        g = j * 4 + sub
                        nc.vector.scalar_tensor_tensor(
                            ob_q[:, ji * 1024 + sub * D
                                 : ji * 1024 + (sub + 1) * D],
                            halves[sub][:, sub % 2, 0:D],
                            r4[:, sub : sub + 1],
                            pe_sb[:, g * D : (g + 1) * D],
                            MUL,
                            ADD,
                        )
                    nc.vector.reciprocal(r4[:, 2:4], psB[:, :, 256])
                    for i, sub in enumerate(range(lo, 4)):
                        nc.scalar.activation(
                            un_q[:, ji, i * D : (i + 1) * D],
                            halves[sub][:, sub % 2, 0:D],
                            Cp, scale=r4[:, sub : sub + 1],
                        )
                    for sub in range(2, lo):
                        g = j * 4 + sub
                        nc.vector.scalar_tensor_tensor(
                            ob_q[:, ji * 1024 + sub * D
                                 : ji * 1024 + (sub + 1) * D],
                            halves[sub][:, sub % 2, 0:D],
                            r4[:, sub : sub + 1],
                            pe_sb[:, g * D : (g + 1) * D],
                            MUL,
                            ADD,
                        )
                # fused PE-add (Pool engine; Vector for the last quad so the
                # tail never waits on a slow Pool add) + out-DMA per (b,j)
                obv = ob_q[:].rearrange("p (g x) -> p g x", g=2)
                pev = pe_sb[:, j0 * 1024 : (j0 + 2) * 1024].rearrange(
                    "p (g x) -> p g x", g=2)
                for ji in range(2):
                    if n_act:
                        eng = nc.vector if qi == len(quads) - 1 else nc.gpsimd
                        eng.tensor_tensor(
                            obv[:, ji : ji + 1, lo * D :],
                            un_q[:, ji : ji + 1, : n_act * D],
                            pev[:, ji : ji + 1, lo * D :],
                            ADD,
                        )
                    nc.sync.dma_start(
                        out_v2[b, :, j0 + ji : j0 + ji + 1],
                        obv[:, ji : ji + 1],
                    )

    nc.compile()
    return nc


def kernel(embeddings, durations, ranges, t_out):
    assert int(t_out) == T_OUT
    emb = np.asarray(embeddings, dtype=np.float32)
    dur = np.asarray(durations, dtype=np.float32)[:, :, 0]
    rng = np.asarray(ranges, dtype=np.float32)[:, :, 0]

    # ---- host preprocessing: O(B*T_in) scalars + window selection ----
    c = np.cumsum(dur, axis=1, dtype=np.float32) - 0.5 * dur   # (B, T_IN)
    a = rng.astype(np.float32) ** -2
    reach = np.sqrt(TH) / np.sqrt(a)

    # window starts: tokens whose gaussian reaches into chunk j
    starts = np.zeros((B, NJ), np.int32)
    span_max = 1
    for b in range(B):
        lo_r, hi_r = c[b] - reach[b], c[b] + reach[b]
        for j in range(NJ):
            qual = np.nonzero((lo_r <= CW * j + CW - 1) & (hi_r >= CW * j))[0]
            if len(qual):
                span_max = max(span_max, int(np.ceil((qual[-1] - qual[0] + 2) / KW)))
                starts[b, j] = qual[0]
            else:
                starts[b, j] = 0
    nkc = span_max
    usable = KW * nkc - 1          # last row of the last k-chunk carries eps
    starts = np.minimum(starts, T_IN - usable)
    # coverage assert (windows are contiguous token ranges)
    for b in range(B):
        lo_r, hi_r = c[b] - reach[b], c[b] + reach[b]
        for j in range(NJ):
            qual = np.nonzero((lo_r <= CW * j + CW - 1) & (hi_r >= CW * j))[0]
            if len(qual):
                assert starts[b, j] <= qual[0] and qual[-1] < starts[b, j] + usable

    # gathered per-window tensors
    ea = np.ones((B, T_IN, 257), BF16)
    ea[:, :, :256] = emb.astype(BF16)
    kidx = starts[:, None, :, None] + (
        np.arange(nkc)[None, :, None, None] * KW + np.arange(KW)[None, None, None, :]
    )  # (B, nkc, NJ, KW); the last slot is overwritten by the eps row below
    kidx = np.minimum(kidx, T_IN - 1)
    bidx = np.arange(B)[:, None, None, None]
    eg = ea[bidx, kidx]                       # (B, nkc, NJ, KW, 257)
    eg = eg.transpose(0, 3, 1, 2, 4).copy()   # (B, KW, nkc, NJ, 257)
    cg = c[bidx, kidx]                        # (B, nkc, NJ, KW)
    ag = a[bidx, kidx]
    jgrid = np.arange(NJ, dtype=np.float32)[None, None, :, None] * CW
    sqa_g = np.sqrt(ag)
    sqa = sqa_g.transpose(3, 0, 1, 2).astype(np.float32)                 # (KW,B,nkc,NJ)
    nsqac = (sqa_g * (jgrid - cg)).transpose(3, 0, 1, 2).astype(np.float32)
    # sacrificial eps row: w~ = kappa*exp(-(0*t + sqrt(-ln eps))^2) =
    # kappa*eps for all t (kappa = 2/sqrt(pi) from Derivative_Erf cancels
    # against the same factor in the numerator weights)
    eg[:, 127, -1, :, :] = 0.0
    eg[:, 127, -1, :, 256] = 1.0
    sqa[127, :, -1, :] = 0.0
    nsqac[127, :, -1, :] = np.float32(np.sqrt(-np.log(EPS)))

    # constants: permuted iota (col f of W is frame t' = 4*(f%128) + f//128)
    f = np.arange(CW)
    tperm = (4 * (f % 128) + f // 128).astype(np.float32)
    iota = np.broadcast_to(tperm, (128, CW)).copy()
    # PE in the same permuted layout: tile[q, j*1024 + sub*256 + d] = PE[512j+4q+sub, d]
    pe = _pe_table().reshape(NJ, 128, 4, D).transpose(1, 0, 2, 3).reshape(128, -1)
    pe = pe.astype(BF16)

    key = nkc
    if key not in _CACHE:
        _CACHE[key] = _build(nkc)
    nc = _CACHE[key]

    NP = BL * nkc * NJ
    in_maps = []
    for i in range(N_CORES):
        bs = slice(i * BL, (i + 1) * BL)
        par = np.concatenate(
            [
                sqa[:, bs].reshape(128, NP),
                nsqac[:, bs].reshape(128, NP),
                iota,
            ],
            axis=1,
        ).astype(np.float32)
        in_maps.append({
            "eg": np.ascontiguousarray(eg[bs]),
            "par": np.ascontiguousarray(par),
            "pe": pe,
        })

    res = run_bass_kernel_spmd(nc, in_maps, core_ids=list(range(N_CORES)))
    out = np.concatenate([r["out"] for r in res.results], axis=0)
    return out.astype(np.float32)
